# revision 14
# baseline (speedup 1.0000x reference)
"""Trainium2 Bass kernel for the ALayer problem (v3: 2-tap-packed streams).

Math (per image):
  y   = sigmoid(fc_w2 @ relu(fc_w1 @ mean_hw(x)))          # [576] channel attn
  A   = sigmoid(conv3x3(relu(conv3x3(x, se_w1)), se_w2))   # [H,W] spatial attn
  out[o,l] = A[l] * sum_{c,t} (weight[o,c,t] * y[c*9+t]) * xpad[c, l+dt]

Strategy: data-parallel, 2 images per core.  Per image the padded x is kept
TWICE on SBUF partitions (xpr: 0-63 unshifted "xA", 64-127 shifted-2-rows
"xB"), so one matmul contracts TWO conv taps (di=0 via xA rows + di=2 via xB
rows); di=1 comes from a 64-row matmul on xA with a +1 row slice.  The SE
conv1 rides in spare lhsT columns of the same matmuls, so main conv + conv1
cost 6 matmul streams per image-tile instead of 18.  conv2 packs its di taps
into partition groups of relu1 (3 row-shifted copies, 32 partitions per
di covering both images block-diagonally) -> 3 matmuls per tile, producing
A_img0 replicated on psum partitions 0-63 and A_img1 on 64-127.  Channel
attention y is folded into the main-conv weights via per-partition scales.
All matmul operands are bf16.
"""

import numpy as np

try:
    import concourse.bass as bass
except ImportError:  # pragma: no cover
    import sys

    sys.path.insert(0, "/opt/trn_rl_repo")
    import concourse.bass as bass

import concourse.mybir as mybir
from concourse import bacc
from concourse.bass_utils import run_bass_kernel_spmd
from concourse.tile import TileContext

F32 = mybir.dt.float32
BF16 = mybir.dt.bfloat16
AF = mybir.ActivationFunctionType
ALU = mybir.AluOpType

B, C, H, W = 16, 64, 128, 128
N_CORES = 8
BPC = B // N_CORES  # images per core = 2
NT = H // 4  # 32 spatial tiles of 4 image rows (512 px) each

_CACHED = {}
TRACE = False


def _build_nc():
    nc = bacc.Bacc(None, target_bir_lowering=False, debug=False)
    x_ext = nc.declare_dram_parameter("x", [BPC, C, H, W], F32, isOutput=False)
    wS_ext = nc.declare_dram_parameter("wS", [128, 3, 80], F32, isOutput=False)
    wT_ext = nc.declare_dram_parameter("wT", [64, 3, 80], F32, isOutput=False)
    w2_ext = nc.declare_dram_parameter("w2blk", [96, 3, 128], F32, isOutput=False)
    fc1_ext = nc.declare_dram_parameter("fc1t", [128, 4], F32, isOutput=False)
    fc2_ext = nc.declare_dram_parameter("fc2t", [4, 576], F32, isOutput=False)
    out_ext = nc.declare_dram_parameter("out", [BPC, C, H, W], F32, isOutput=True)

    xv = x_ext[:].rearrange("b c h w -> (b c) h w")  # [128, 128, 128]
    ov = out_ext[:].rearrange("b c h w -> (b c) h w")

    with TileContext(nc) as tc:
        with (
            tc.tile_pool(name="persist", bufs=1) as pp,
            tc.tile_pool(name="stage", bufs=3) as sp,
            tc.tile_pool(name="asb", bufs=2) as ap_pool,
            tc.tile_pool(name="io", bufs=3) as iop,
            tc.tile_pool(name="r1t", bufs=2) as r1p,
            tc.tile_pool(name="psA", bufs=1, space="PSUM") as psA,
            tc.tile_pool(name="psM0", bufs=3, space="PSUM") as psM0,
            tc.tile_pool(name="psM1", bufs=3, space="PSUM") as psM1,
            tc.tile_pool(name="psY", bufs=1, space="PSUM") as psY,
        ):
            # ---- persistent SBUF tiles
            # xpr: per image, partitions 0-63 = xA (x padded, row h holds
            # x[h-1]), partitions 64-127 = xB (row h holds x[h+1]).
            xpr = [pp.tile([128, H + 2, W + 2], BF16, name=f"xpr{i}") for i in range(2)]
            # r1all: relu1 of both images, 3 row-shifted groups of 32
            # partitions (16 per image): block g partitions [32g, 32g+32),
            # img0 at +0, img1 at +16; content relu1[h] at storage row h+2-g.
            r1all = pp.tile([128, H + 2, W + 2], BF16)
            wSf = pp.tile([128, 3, 80], F32)
            wSb = pp.tile([128, 3, 80], BF16)
            wTf = pp.tile([64, 3, 80], F32)
            wTb = pp.tile([64, 3, 80], BF16)
            w2f = pp.tile([96, 3, 128], F32)
            w2b = pp.tile([96, 3, 128], BF16)
            fc1f = pp.tile([128, 4], F32)
            fc1b = pp.tile([128, 4], BF16)
            fc2f = pp.tile([4, 576], F32)
            fc2b = pp.tile([4, 576], BF16)
            # per-(img, dj) effective lhsT tiles: cols 0-63 main (y-scaled),
            # cols 64-79 se1 (relu lands at psum partitions 64-79).
            weffS = [
                [pp.tile([128, 80], BF16, name=f"wS{i}{dj}") for dj in range(3)]
                for i in range(2)
            ]
            weffT = [
                [pp.tile([64, 80], BF16, name=f"wT{i}{dj}") for dj in range(3)]
                for i in range(2)
            ]
            sums = pp.tile([128, 8], F32)
            sumtot = pp.tile([128, 1], F32)
            mean_b = pp.tile([128, 1], BF16)
            y1sb = pp.tile([4, 2], BF16)
            # y scale tiles: yscl[i][p, dj] = y_i[c, 3*di+dj] with di=0 for
            # p<64 and di=2 for p>=64; ymid[i][c, dj] = y_i[c, 3+dj].
            yscl = [pp.tile([128, 3], F32, name=f"yscl{i}") for i in range(2)]
            ymid = [pp.tile([64, 3], F32, name=f"ymid{i}") for i in range(2)]

            # ---- border memsets (gpsimd; off every other engine's path)
            for i in range(2):
                nc.gpsimd.memset(xpr[i][:, 0:1, :], 0.0)  # xA top pad / xB row0 pre
                nc.gpsimd.memset(xpr[i][:, H + 1 : H + 2, :], 0.0)  # xA bottom pad
                nc.gpsimd.memset(xpr[i][64:128, H - 1 : H + 1, :], 0.0)  # xB tail
                nc.gpsimd.memset(xpr[i][:, :, 0:1], 0.0)
                nc.gpsimd.memset(xpr[i][:, :, W + 1 : W + 2], 0.0)
            nc.gpsimd.memset(r1all[0:32, 1:2, :], 0.0)  # g0: relu1[-1] = 0
            nc.gpsimd.memset(r1all[64:96, H : H + 1, :], 0.0)  # g2: relu1[H] = 0
            nc.gpsimd.memset(r1all[0:96, :, 0:1], 0.0)
            nc.gpsimd.memset(r1all[0:96, :, W + 1 : W + 2], 0.0)

            # ---- parameter loads + bf16 casts (engines idle during x-load)
            for ext, ft, bt in (
                (fc1_ext, fc1f, fc1b),
                (fc2_ext, fc2f, fc2b),
                (wS_ext, wSf, wSb),
                (wT_ext, wTf, wTb),
                (w2_ext, w2f, w2b),
            ):
                nc.sync.dma_start(out=ft[:], in_=ext[:])
                nc.scalar.activation(out=bt[:], in_=ft[:], func=AF.Copy)
            # static se1 columns of the chain lhsT tiles
            for i in range(2):
                for dj in range(3):
                    nc.vector.tensor_copy(weffS[i][dj][:, 64:80], wSb[:, dj, 64:80])
                    nc.vector.tensor_copy(weffT[i][dj][:, 64:80], wTb[:, dj, 64:80])

            # ---- x load (f32) -> cast to padded bf16 xA halves + row sums;
            # xB halves are shifted bf16 copies of xA.
            for j in range(8):
                st = sp.tile([128, 16, W], F32, tag="xstage")
                nc.sync.dma_start(out=st[:], in_=xv[:, 16 * j : 16 * j + 16, :])
                r0, r1 = 1 + 16 * j, 17 + 16 * j
                nc.vector.scalar_tensor_tensor(
                    out=xpr[0][0:64, r0:r1, 1 : W + 1],
                    in0=st[0:64],
                    scalar=0.0,
                    in1=st[0:64],
                    op0=ALU.add,
                    op1=ALU.bypass,
                    accum_out=sums[0:64, j : j + 1],
                )
                nc.scalar.activation(
                    out=xpr[1][0:64, r0:r1, 1 : W + 1],
                    in_=st[64:128],
                    func=AF.Copy,
                    accum_out=sums[64:128, j : j + 1],
                )
                # xB[h] = xA[h+2]: per chunk, copy the just-cast rows down 2.
                lo = max(16 * j - 1, 0)
                hi = min(16 * j + 15, H - 1)  # xB rows [lo, hi); row H-1 memset
                cp_eng = nc.vector if j % 2 == 0 else nc.gpsimd
                cp2_eng = nc.gpsimd if j % 2 == 0 else nc.vector
                cp_eng.tensor_copy(
                    xpr[0][64:128, lo:hi, :], xpr[0][0:64, lo + 2 : hi + 2, :]
                )
                cp2_eng.tensor_copy(
                    xpr[1][64:128, lo:hi, :], xpr[1][0:64, lo + 2 : hi + 2, :]
                )
                # HAM warm-up: keep the PE p-state high through the load
                if j >= 2:
                    wp = psY.tile([128, 4, W], F32, tag="y", name=f"warm{j}")
                    nc.tensor.matmul(
                        wp[:],
                        lhsT=xpr[0][0:64, r0 : r0 + 1, 0:128],
                        rhs=xpr[0][0:64, r0 : r0 + 4, 1 : W + 1],
                        start=True,
                        stop=True,
                    )

            # ---- channel-attention chain -> per-partition weight scales
            nc.vector.reduce_sum(out=sumtot[:], in_=sums[:], axis=mybir.AxisListType.X)
            nc.vector.tensor_copy(mean_b[:], sumtot[:])  # 1/HW folded into fc1t
            ytile = psY.tile([128, 16], F32, tag="y", name="ytile")
            for i in range(2):
                nc.tensor.matmul(
                    ytile[0:4, 12 + i : 13 + i],
                    lhsT=fc1b[64 * i : 64 * i + 64, :],
                    rhs=mean_b[64 * i : 64 * i + 64, :],
                    start=True,
                    stop=True,
                )
                nc.scalar.activation(
                    out=y1sb[:, i : i + 1], in_=ytile[0:4, 12 + i : 13 + i], func=AF.Relu
                )
            # warm matmul to bridge the y-chain PE gap
            wpy = psY.tile([128, 4, W], F32, tag="y", name="warmy")
            nc.tensor.matmul(
                wpy[:],
                lhsT=xpr[0][0:64, 1:2, 0:128],
                rhs=xpr[0][0:64, 1:5, 1 : W + 1],
                start=True,
                stop=True,
            )
            fc2v = fc2b[:].rearrange("p (c t) -> p c t", c=64, t=9)
            for i in range(2):
                for dj in range(3):
                    # di=0 -> yscl rows 0-63; di=2 -> rows 64-127; di=1 -> ymid
                    nc.tensor.matmul(
                        ytile[0:64, 3 * i + dj : 3 * i + dj + 1],
                        lhsT=fc2v[:, :, dj],
                        rhs=y1sb[:, i : i + 1],
                        start=True,
                        stop=True,
                    )
                    nc.tensor.matmul(
                        ytile[64:128, 3 * i + dj : 3 * i + dj + 1],
                        lhsT=fc2v[:, :, 6 + dj],
                        rhs=y1sb[:, i : i + 1],
                        start=True,
                        stop=True,
                    )
                    nc.tensor.matmul(
                        ytile[0:64, 6 + 3 * i + dj : 6 + 3 * i + dj + 1],
                        lhsT=fc2v[:, :, 3 + dj],
                        rhs=y1sb[:, i : i + 1],
                        start=True,
                        stop=True,
                    )
            for i in range(2):
                nc.scalar.activation(
                    out=yscl[i][:, :], in_=ytile[:, 3 * i : 3 * i + 3], func=AF.Sigmoid
                )
                nc.scalar.activation(
                    out=ymid[i][:, :],
                    in_=ytile[0:64, 6 + 3 * i : 9 + 3 * i],
                    func=AF.Sigmoid,
                )
            # y-scaled main-conv weight columns (DVE + ACT in parallel)
            for dj in range(3):
                nc.vector.tensor_scalar_mul(
                    weffS[0][dj][:, 0:64], wSb[:, dj, 0:64], yscl[0][:, dj : dj + 1]
                )
                nc.scalar.activation(
                    out=weffS[1][dj][:, 0:64],
                    in_=wSb[:, dj, 0:64],
                    func=AF.Copy,
                    scale=yscl[1][:, dj : dj + 1],
                )
                nc.vector.tensor_scalar_mul(
                    weffT[0][dj][:, 0:64], wTb[:, dj, 0:64], ymid[0][:, dj : dj + 1]
                )
                nc.scalar.activation(
                    out=weffT[1][dj][:, 0:64],
                    in_=wTb[:, dj, 0:64],
                    func=AF.Copy,
                    scale=ymid[1][:, dj : dj + 1],
                )

            # ---- main loop: 12 fused main+conv1 matmuls, then lagged
            # conv2/epilogue (conv2 of tile k needs relu1 rows from tile k+1).
            lag = []

            def epilogue(k, osb):
                r0 = 4 * k
                psa = psA.tile([128, 4, W], F32, tag="A")
                for dj in range(3):
                    nc.tensor.matmul(
                        psa[:],
                        lhsT=w2b[:, dj, :],
                        rhs=r1all[0:96, r0 + 1 : r0 + 5, dj : dj + W],
                        start=(dj == 0),
                        stop=(dj == 2),
                    )
                asb = ap_pool.tile([128, 4, W], F32, tag="Asb")
                nc.scalar.activation(out=asb[:], in_=psa[:], func=AF.Sigmoid)
                # A-multiply in place on the sbuf staging (psum long freed)
                nc.vector.tensor_mul(osb[0:64], osb[0:64], asb[0:64])
                nc.vector.tensor_mul(osb[64:128], osb[64:128], asb[64:128])
                out_eng = nc.sync if k % 2 == 0 else nc.scalar
                out_eng.dma_start(out=ov[:, r0 : r0 + 4, :], in_=osb[:])

            for k in range(NT):
                r0 = 4 * k
                pm0 = psM0.tile([128, 4, W], F32, tag="m0")
                pm1 = psM1.tile([128, 4, W], F32, tag="m1")
                for i, pm in ((0, pm0), (1, pm1)):
                    for dj in range(3):
                        nc.tensor.matmul(
                            pm[0:80],
                            lhsT=weffS[i][dj][:],
                            rhs=xpr[i][:, r0 : r0 + 4, dj : dj + W],
                            start=(dj == 0),
                            stop=False,
                        )
                        nc.tensor.matmul(
                            pm[0:80],
                            lhsT=weffT[i][dj][:],
                            rhs=xpr[i][0:64, r0 + 1 : r0 + 5, dj : dj + W],
                            start=False,
                            stop=(dj == 2),
                        )
                # evict both psum main regions to the sbuf output staging
                # immediately so the psum banks recycle without waiting on
                # the (lagged) A-multiply.
                osb = iop.tile([128, 4, W], F32, tag="osb")
                nc.scalar.activation(out=osb[0:64], in_=pm0[0:64], func=AF.Copy)
                nc.vector.tensor_copy(osb[64:128], pm1[0:64])
                # issue the lagged epilogue BEFORE this tile's relu evictions
                # (whole-tile dep tracking on r1all).
                if len(lag) == 2:
                    epilogue(*lag.pop(0))
                # relu1 eviction into group g0 (+2 row offset), then shifted
                # bf16 copies for g1/g2 (one 32-partition op covers both imgs).
                # img1's g0 slot (partitions 16-31) is not a legal compute-
                # engine base, so it goes ACT -> staging tile -> DMA scatter.
                nc.scalar.activation(
                    out=r1all[0:16, r0 + 2 : r0 + 6, 1 : W + 1],
                    in_=pm0[64:80],
                    func=AF.Relu,
                )
                r1tmp = r1p.tile([16, 4, W], BF16, tag="r1tmp")
                nc.scalar.activation(out=r1tmp[:], in_=pm1[64:80], func=AF.Relu)
                nc.gpsimd.dma_start(
                    out=r1all[16:32, r0 + 2 : r0 + 6, 1 : W + 1], in_=r1tmp[:]
                )
                nc.vector.tensor_copy(
                    r1all[32:64, r0 + 1 : r0 + 5, 1 : W + 1],
                    r1all[0:32, r0 + 2 : r0 + 6, 1 : W + 1],
                )
                nc.vector.tensor_copy(
                    r1all[64:96, r0 : r0 + 4, 1 : W + 1],
                    r1all[0:32, r0 + 2 : r0 + 6, 1 : W + 1],
                )
                lag.append((k, osb))
            while lag:
                epilogue(*lag.pop(0))

    nc.finalize()
    return nc


def _prep_params(weight, se_w1, se_w2, fc_w1, fc_w2):
    # weight [64, 64, 3, 3] -> w[o, c, di, dj]; se_w1 [16, 64, 3, 3]
    wS = np.zeros((128, 3, 80), np.float32)
    wT = np.zeros((64, 3, 80), np.float32)
    for dj in range(3):
        wS[0:64, dj, 0:64] = weight[:, :, 0, dj].T  # rows c (di=0) -> cols o
        wS[64:128, dj, 0:64] = weight[:, :, 2, dj].T  # rows c (di=2)
        wS[0:64, dj, 64:80] = se_w1[:, :, 0, dj].T  # cols s
        wS[64:128, dj, 64:80] = se_w1[:, :, 2, dj].T
        wT[:, dj, 0:64] = weight[:, :, 1, dj].T
        wT[:, dj, 64:80] = se_w1[:, :, 1, dj].T
    w2 = np.zeros((96, 3, 128), np.float32)
    s2 = se_w2.reshape(16, 3, 3)  # [s, di, dj]
    for g in range(3):
        for dj in range(3):
            w2[32 * g : 32 * g + 16, dj, 0:64] = s2[:, g, dj][:, None]
            w2[32 * g + 16 : 32 * g + 32, dj, 64:128] = s2[:, g, dj][:, None]
    fc1 = np.zeros((128, 4), np.float32)
    f1 = fc_w1.T.astype(np.float32) / float(H * W)
    fc1[:64] = f1
    fc1[64:] = f1
    fc2 = fc_w2.T.astype(np.float32)  # [4, 576]
    return wS, wT, w2, fc1, fc2


def kernel(x, weight, se_w1, se_w2, fc_w1, fc_w2):
    x = np.ascontiguousarray(x, np.float32)
    wS, wT, w2, fc1, fc2 = _prep_params(
        np.asarray(weight, np.float32),
        np.asarray(se_w1, np.float32),
        np.asarray(se_w2, np.float32),
        np.asarray(fc_w1, np.float32),
        np.asarray(fc_w2, np.float32),
    )
    if "nc" not in _CACHED:
        _CACHED["nc"] = _build_nc()
    nc = _CACHED["nc"]
    in_maps = [
        {
            "x": np.ascontiguousarray(x[BPC * i : BPC * i + BPC]),
            "wS": wS,
            "wT": wT,
            "w2blk": w2,
            "fc1t": fc1,
            "fc2t": fc2,
        }
        for i in range(N_CORES)
    ]
    res = run_bass_kernel_spmd(
        nc, in_maps, core_ids=list(range(N_CORES)), trace=TRACE
    )
    if TRACE:
        print(f"HW exec time: {res.exec_time_ns} ns")
        _CACHED["res"] = res
    out = np.concatenate([r["out"] for r in res.results], axis=0)
    return out.reshape(B, C, H, W).astype(np.float32)
